# revision 9
# baseline (speedup 1.0000x reference)
"""Trainium2 Bass kernel for a single DeBERTa-style attention head.

Problem shapes (hardcoded):
  B=8, S=2048, E=768(n_embed), H=64(head)
  q = I @ Wq + bq ; k = x @ Wk + bk ; v = x @ Wv + bv
  w = (q @ k^T) / sqrt(E) ; w = where(mask==0, -1e9, w)
  scores = softmax(w, axis=-1) ; out = scores @ v

Sharding: data-parallel over batch B across the 8 NeuronCores.

v2 design (v1 was ~97.6us, DMA-front-loaded + PE-inefficient):
  * Host pre-casts I/x to bf16 -> HBM input traffic 16MB -> 10MB per core;
    mask stays uint8, expanded to bf16 by cast-DMA, streamed in 4-ki chunks.
  * exp on ACT is the hard floor (~32us for 4.2M logits at 1/lane/cycle);
    the schedule keeps ACT fed from ~6us: load order is IT0 XT0 IT1 XT1
    XT2 XT3 mask(0:4) IT2 IT3 mask(4:16), with k-projections for the back
    half split out (they only need x^T) so score pairs 4-7 q-half-0 can run
    before I^T finishes loading.
  * q/k projections col-tiled: lhsT=[Wq|Wk] as two concurrent 64-col PE
    tiles (q streams I^T, k streams x^T) -> one 512-cycle slot per (nb, ei);
    one full-lane DVE copy applies both biases from a [128,1] bias vector.
  * qT/kT duplicated into the opposite partition halves (DUP = [kT;qT]) via
    on-chip SBUF->SBUF DMA so score matmuls run 2x row-tiled (K=64): two ki
    chunks compute concurrently per 512-col slot.
  * exp writes straight into the sT tile; the mask multiply is an in-place
    DVE tensor_tensor (2x mode), emitted in mask-arrival order so the DVE
    stream never head-of-line blocks.
  * v projection packs 8 seq-chunks per PSUM bank (2 banks), one rank-1
    bias matmul + one DVE copy per bank.
  * ctx accumulation split per q-half so it starts as soon as the first
    half of each sT row-block is masked; epilogue in two halves so the
    output DMA overlaps the last ctx matmuls; output bf16, host upcasts.
"""

import math
from contextlib import ExitStack

import numpy as np

import concourse.bass as bass
import concourse.tile as tile
import concourse.mybir as mybir
from concourse import bacc
from concourse.bass_utils import run_bass_kernel_spmd

B, S, E, H = 8, 2048, 768, 64
N_CORES = 8
SC = S // 128   # 16 seq chunks
EC = E // 128   # 6 embed chunks
SCALE = 1.0 / math.sqrt(E)

F32 = mybir.dt.float32
BF16 = mybir.dt.bfloat16
U8 = mybir.dt.uint8
AF = mybir.ActivationFunctionType
ALU = mybir.AluOpType

_cache = {}


def _build_program():
    nc = bacc.Bacc("TRN2", target_bir_lowering=False, debug=False)

    dIT = nc.dram_tensor("IT", [E, S], BF16, kind="ExternalInput")
    dXT = nc.dram_tensor("XT", [E, S], BF16, kind="ExternalInput")
    dmT = nc.dram_tensor("maskT", [S, S], U8, kind="ExternalInput")
    dWqk = nc.dram_tensor("Wqk", [E, 128], BF16, kind="ExternalInput")
    dWv = nc.dram_tensor("Wv", [E, H], BF16, kind="ExternalInput")
    dbqk = nc.dram_tensor("bqk", [128, 1], F32, kind="ExternalInput")
    dbv = nc.dram_tensor("bv512", [1, 512], BF16, kind="ExternalInput")
    dout = nc.dram_tensor("out", [S, H], BF16, kind="ExternalOutput")

    with tile.TileContext(nc) as tc, ExitStack() as ctx:
        singles = ctx.enter_context(tc.tile_pool(name="singles", bufs=1))

        IT = singles.tile([128, EC, S], BF16, tag="IT")
        XT = singles.tile([128, EC, S], BF16, tag="XT")

        def load_chunk(dst, src, nb):
            # HWDGE (sync) path: ~0.6us first-byte and a separate issue ring
            # from the gpsimd cast-DMAs carrying the mask
            lo, hi = nb * 512, (nb + 1) * 512
            nc.sync.dma_start(
                out=dst[:, :, lo:hi],
                in_=src.ap()[:, lo:hi].rearrange("(ec p) s -> p ec s", p=128),
            )

        # q-side needs I^T, k/v-side needs x^T; scores for the back half
        # only need x^T early, so x^T chunks 2,3 jump the queue
        load_chunk(IT, dIT, 0)
        load_chunk(XT, dXT, 0)
        load_chunk(IT, dIT, 1)
        load_chunk(XT, dXT, 1)
        load_chunk(XT, dXT, 2)
        load_chunk(XT, dXT, 3)

        load_chunk(IT, dIT, 2)
        load_chunk(IT, dIT, 3)

        # mask fully resident (no pool slot gating on the cast-DMA stream),
        # loaded in 4-ki chunks on the gpsimd (SWDGE) ring which runs in
        # parallel with the sync-ring I/x stream above
        masks = {}
        for mi in range(4):
            masks[mi] = singles.tile(
                [128, 4, S], BF16, name=f"mask{mi}", tag=f"mask{mi}"
            )
            nc.gpsimd.dma_start(
                out=masks[mi],
                in_=dmT.ap()[mi * 512:(mi + 1) * 512, :].rearrange(
                    "(t p) q -> p t q", p=128
                ),
            )

        ones_row = singles.tile([1, 512], BF16, tag="ones")
        nc.vector.memset(ones_row, 1.0)

        wqk_sb = singles.tile([128, EC, 128], BF16, tag="Wqk")
        nc.sync.dma_start(
            out=wqk_sb, in_=dWqk.ap().rearrange("(ec p) h -> p ec h", p=128)
        )
        wv_sb = singles.tile([128, EC, H], BF16, tag="Wv")
        nc.sync.dma_start(
            out=wv_sb, in_=dWv.ap().rearrange("(ec p) h -> p ec h", p=128)
        )
        bqk_sb = singles.tile([128, 1], F32, tag="bqk")
        nc.sync.dma_start(out=bqk_sb, in_=dbqk.ap())
        bv_sb = singles.tile([1, 512], BF16, tag="bv512")
        nc.sync.dma_start(out=bv_sb, in_=dbv.ap())

        # QK rows 0:64 = qT, rows 64:128 = kT.  DUP is the partition-swapped
        # copy (rows 0:64 = kT, 64:128 = qT) so both score row-tiles find
        # their operands at the right base partition.
        QK = singles.tile([128, S], BF16, tag="QK")
        DUP = singles.tile([128, S], BF16, tag="DUP")
        vA = singles.tile([128, SC, 66], BF16, tag="vA")
        nc.vector.memset(vA[:, :, H:H + 1], 1.0)

        sp = ctx.enter_context(tc.tile_pool(name="sp", bufs=16))
        psw = ctx.enter_context(tc.tile_pool(name="psw", bufs=2, space="PSUM"))
        outp = ctx.enter_context(tc.tile_pool(name="outp", bufs=1))

        def emit_proj(ps2, nb):
            """Col-tiled q|k projection for columns nb*512:(nb+1)*512."""
            lo = nb * 512
            ps = ps2.tile([128, 512], F32, tag="pqk")
            for ei in range(EC):
                # q and k are separate col-tile accumulation groups on the
                # same bank; each clears its own partition range at ei=0
                nc.tensor.matmul(
                    ps[0:64, :],
                    lhsT=wqk_sb[:, ei, 0:64],
                    rhs=IT[:, ei, lo:lo + 512],
                    start=(ei == 0),
                    stop=(ei == EC - 1),
                    skip_group_check=True,
                )
                nc.tensor.matmul(
                    ps[64:128, :],
                    lhsT=wqk_sb[:, ei, 64:128],
                    rhs=XT[:, ei, lo:lo + 512],
                    start=(ei == 0),
                    stop=(ei == EC - 1),
                    skip_group_check=True,
                )
            nc.vector.tensor_scalar(
                QK[:, lo:lo + 512], ps, bqk_sb, None, ALU.add
            )
            # partition-swapped copies via on-chip DMA (compute engines
            # cannot move data across partitions; DMA can, cheaply)
            nc.sync.dma_start(out=DUP[0:64, lo:lo + 512], in_=QK[64:128, lo:lo + 512])
            nc.sync.dma_start(out=DUP[64:128, lo:lo + 512], in_=QK[0:64, lo:lo + 512])

        def emit_halfproj(half, nb):
            """Uncoupled q-only (half=0) or k-only (half=1) projection for
            the back columns, so the k side can run before I^T arrives."""
            lo = nb * 512
            rlo = 64 * half
            ps = psw.tile([128, 1024], F32, tag="w")
            src = IT if half == 0 else XT
            for ei in range(EC):
                nc.tensor.matmul(
                    ps[rlo:rlo + 64, 0:512],
                    lhsT=wqk_sb[:, ei, rlo:rlo + 64],
                    rhs=src[:, ei, lo:lo + 512],
                    start=(ei == 0),
                    stop=(ei == EC - 1),
                )
            nc.vector.tensor_scalar(
                QK[rlo:rlo + 64, lo:lo + 512],
                ps[rlo:rlo + 64, 0:512],
                bqk_sb[rlo:rlo + 64],
                None,
                ALU.add,
            )
            nc.sync.dma_start(
                out=DUP[64 - rlo:128 - rlo, lo:lo + 512],
                in_=QK[rlo:rlo + 64, lo:lo + 512],
            )

        def emit_v_bank(psv, vb):
            """v projection for seq chunks 8*vb..8*vb+7 packed in one bank."""
            ps = psv.tile([128, 512], F32, tag="pv")
            for j in range(8):
                kb = vb * 8 + j
                for ei in range(EC):
                    nc.tensor.matmul(
                        ps[:, j * 64:(j + 1) * 64],
                        lhsT=XT[:, ei, kb * 128:(kb + 1) * 128],
                        rhs=wv_sb[:, ei, :],
                        start=(j == 0 and ei == 0),
                        stop=False,
                    )
            # one rank-1 bias matmul covers all 8 chunks (bv tiled 8x)
            nc.tensor.matmul(
                ps, lhsT=ones_row[:, 0:128], rhs=bv_sb, start=False, stop=True
            )
            nc.vector.tensor_copy(vA[:, vb * 8:(vb + 1) * 8, 0:H], ps)

        sTs = {}

        def emit_wexp(t, hh, split=False):
            """Row-tiled scores + exp for ki pair (2t, 2t+1), q-half hh.

            Two K=64 tiles run concurrently on the PE: tile (0,0) computes
            ki_a from DUP/QK partitions 0:64, tile (64,0) computes ki_b from
            partitions 64:128.  exp writes straight into the sT tiles; the
            mask multiply is emitted separately (emit_mult) in mask-arrival
            order.  split=True emits per-512-col exps so the first ACT op
            isn't gated on the second rhs chunk's projection."""
            ki_a, ki_b = 2 * t, 2 * t + 1
            qlo = hh * 1024
            if ki_a not in sTs:
                sTs[ki_a] = sp.tile([128, S], BF16, name=f"sT{ki_a}", tag="sT")
                sTs[ki_b] = sp.tile([128, S], BF16, name=f"sT{ki_b}", tag="sT")
            for ki, rlo in ((ki_a, 0), (ki_b, 64)):
                wp = psw.tile([128, 1024], F32, tag="w")
                src = DUP if rlo == 0 else QK
                mov = QK if rlo == 0 else DUP
                for nb2 in range(2):
                    nc.tensor.matmul(
                        wp[:, nb2 * 512:(nb2 + 1) * 512],
                        lhsT=src[rlo:rlo + 64, ki * 128:(ki + 1) * 128],
                        rhs=mov[rlo:rlo + 64, qlo + nb2 * 512:qlo + (nb2 + 1) * 512],
                        start=True,
                        stop=True,
                    )
                    if split:
                        nc.scalar.activation(
                            sTs[ki][:, qlo + nb2 * 512:qlo + (nb2 + 1) * 512],
                            wp[:, nb2 * 512:(nb2 + 1) * 512],
                            AF.Exp,
                            scale=SCALE,
                        )
                if not split:
                    nc.scalar.activation(
                        sTs[ki][:, qlo:qlo + 1024], wp, AF.Exp, scale=SCALE
                    )

        def emit_mult(t, hh):
            """In-place mask multiply for ki pair (2t, 2t+1), q-half hh."""
            qlo = hh * 1024
            for ki in (2 * t, 2 * t + 1):
                nc.vector.tensor_tensor(
                    sTs[ki][:, qlo:qlo + 1024],
                    sTs[ki][:, qlo:qlo + 1024],
                    masks[ki // 4][:, ki % 4, qlo:qlo + 1024],
                    ALU.mult,
                )

        def emit_ctx(ki, qjs, ctxall):
            sT_sb = sTs[ki]
            for qj in qjs:
                nc.tensor.matmul(
                    ctxall[:, qj, 0:H + 1],
                    lhsT=sT_sb[:, qj * 128:(qj + 1) * 128],
                    rhs=vA[:, ki, 0:H + 1],
                    start=(ki == 0 and qj % 4 == 0),
                    stop=(ki == SC - 1 and qj % 4 == 3),
                )

        with tc.tile_pool(name="ps2", bufs=2, space="PSUM") as ps2, \
             tc.tile_pool(name="psv", bufs=2, space="PSUM") as psv:
            # PE warmup: ~3.5us of rank-1 streams during the initial DMA wait
            # flips the HAM clock-gate to 2.4GHz before real work arrives
            for _ in range(8):
                wt = ps2.tile([128, 512], F32, tag="pqk")
                nc.tensor.matmul(
                    wt, lhsT=ones_row[:, 0:128], rhs=ones_row, start=True, stop=True
                )
            emit_proj(ps2, 0)
            emit_proj(ps2, 1)
            emit_wexp(0, 0, split=True)
            emit_wexp(1, 0, split=True)
            emit_v_bank(psv, 0)
            emit_wexp(2, 0)
            emit_wexp(3, 0)
            emit_halfproj(1, 2)   # kT cols 1024:2048 from x^T (arrives early)
            emit_halfproj(1, 3)
            emit_v_bank(psv, 1)
            emit_wexp(4, 0)
            emit_wexp(5, 0)
            emit_wexp(6, 0)
            emit_wexp(7, 0)

        # prologue PSUM pools closed -> 4 banks free for ctx accumulation
        psctx = ctx.enter_context(tc.tile_pool(name="psctx", bufs=1, space="PSUM"))
        ctxall = psctx.tile([128, SC, 128], F32, tag="ctxall")
        Q07 = tuple(range(8))
        Q8F = tuple(range(8, SC))

        emit_mult(0, 0)
        emit_mult(1, 0)
        emit_ctx(0, Q07, ctxall)
        emit_ctx(1, Q07, ctxall)
        emit_ctx(2, Q07, ctxall)
        emit_ctx(3, Q07, ctxall)
        emit_halfproj(0, 2)   # qT cols 1024:2048 once I^T lands
        emit_halfproj(0, 3)
        emit_wexp(0, 1)
        emit_wexp(1, 1)
        # DVE stream in mask-arrival order: masks 0:4 early, 4:8 after I^T,
        # then 8:12, 12:16; hh1 multiplies trail their exps
        emit_mult(0, 1)
        emit_ctx(0, Q8F, ctxall)
        emit_wexp(2, 1)
        emit_mult(2, 0)
        emit_mult(3, 0)
        emit_ctx(4, Q07, ctxall)
        emit_ctx(5, Q07, ctxall)
        emit_ctx(6, Q07, ctxall)
        emit_ctx(7, Q07, ctxall)
        emit_wexp(3, 1)
        emit_mult(1, 1)
        emit_ctx(1, Q8F, ctxall)
        emit_wexp(4, 1)
        emit_mult(4, 0)
        emit_ctx(8, Q07, ctxall)
        emit_ctx(9, Q07, ctxall)
        emit_mult(2, 1)
        emit_ctx(2, Q8F, ctxall)
        emit_wexp(5, 1)
        emit_mult(5, 0)
        emit_ctx(10, Q07, ctxall)
        emit_ctx(11, Q07, ctxall)
        emit_mult(6, 0)
        emit_mult(7, 0)
        emit_ctx(12, Q07, ctxall)
        emit_ctx(13, Q07, ctxall)
        emit_ctx(14, Q07, ctxall)
        emit_ctx(15, Q07, ctxall)
        emit_wexp(6, 1)
        emit_mult(3, 1)
        emit_ctx(3, Q8F, ctxall)
        emit_mult(4, 1)
        emit_ctx(8, Q8F, ctxall)
        emit_ctx(9, Q8F, ctxall)
        emit_wexp(7, 1)
        emit_mult(5, 1)
        emit_ctx(4, Q8F, ctxall)
        emit_ctx(5, Q8F, ctxall)
        emit_ctx(10, Q8F, ctxall)
        emit_ctx(11, Q8F, ctxall)
        emit_mult(6, 1)
        emit_ctx(6, Q8F, ctxall)
        emit_ctx(7, Q8F, ctxall)
        emit_ctx(12, Q8F, ctxall)
        emit_ctx(13, Q8F, ctxall)
        emit_mult(7, 1)
        emit_ctx(14, Q8F, ctxall)

        # epilogue in two halves so output DMA overlaps the last ctx matmuls
        recip_t = outp.tile([128, SC, 1], F32, tag="recip")
        o_all = outp.tile([128, SC, H], BF16, tag="o")

        def emit_epilogue(qlo, qhi):
            nc.vector.reciprocal(
                recip_t[:, qlo:qhi], ctxall[:, qlo:qhi, H:H + 1]
            )
            rb = bass.AP(
                tensor=recip_t.tensor,
                offset=recip_t.offset + qlo * recip_t.ap[1][0],
                ap=[recip_t.ap[0], [recip_t.ap[1][0], qhi - qlo], [0, H]],
            )
            nc.vector.tensor_tensor(
                o_all[:, qlo:qhi], ctxall[:, qlo:qhi, 0:H], rb, ALU.mult
            )
            nc.sync.dma_start(
                out=dout.ap()[qlo * 128:qhi * 128].rearrange(
                    "(qj p) h -> p qj h", p=128
                ),
                in_=o_all[:, qlo:qhi],
            )

        emit_epilogue(0, 8)
        emit_ctx(15, Q8F, ctxall)
        emit_epilogue(8, SC)

    nc.compile()
    return nc


def get_program():
    if "nc" not in _cache:
        _cache["nc"] = _build_program()
    return _cache["nc"]


def make_in_maps(I, x, mask, Wq, bq, Wk, bk, Wv, bv):
    import ml_dtypes

    BF = ml_dtypes.bfloat16
    I = np.asarray(I, dtype=np.float32)
    x = np.asarray(x, dtype=np.float32)
    mask = np.asarray(mask, dtype=np.int32)

    Wqk = np.concatenate(
        [np.asarray(Wq, np.float32), np.asarray(Wk, np.float32)], axis=1
    ).astype(BF)
    Wv_ = np.asarray(Wv, np.float32).astype(BF)
    bqk = np.concatenate(
        [np.asarray(bq, np.float32), np.asarray(bk, np.float32)]
    ).reshape(128, 1).astype(np.float32)
    bv512 = np.tile(np.asarray(bv, np.float32).reshape(1, H), (1, 8)).astype(BF)

    return [
        {
            "IT": np.ascontiguousarray(I[b].T).astype(BF),
            "XT": np.ascontiguousarray(x[b].T).astype(BF),
            "maskT": np.ascontiguousarray(mask[b].T).astype(np.uint8),
            "Wqk": Wqk, "Wv": Wv_, "bqk": bqk, "bv512": bv512,
        }
        for b in range(B)
    ]


def kernel(I, x, mask, Wq, bq, Wk, bk, Wv, bv):
    nc = get_program()
    in_maps = make_in_maps(I, x, mask, Wq, bq, Wk, bk, Wv, bv)
    res = run_bass_kernel_spmd(nc, in_maps, list(range(N_CORES)))
    out = np.stack([res.results[b]["out"] for b in range(B)], axis=0)
    return out.astype(np.float32)


# revision 11
# speedup vs baseline: 1.2233x; 1.2233x over previous
"""Trainium2 Bass kernel for a single DeBERTa-style attention head.

Problem shapes (hardcoded):
  B=8, S=2048, E=768(n_embed), H=64(head)
  q = I @ Wq + bq ; k = x @ Wk + bk ; v = x @ Wv + bv
  w = (q @ k^T) / sqrt(E) ; w = where(mask==0, -1e9, w)
  scores = softmax(w, axis=-1) ; out = scores @ v

Sharding: data-parallel over batch B across the 8 NeuronCores.

v2 design (v1 was ~97.6us, DMA-front-loaded + PE-inefficient):
  * Host pre-casts I/x to bf16 -> HBM input traffic 16MB -> 10MB per core;
    mask stays uint8, expanded to bf16 by cast-DMA, streamed in 4-ki chunks.
  * exp on ACT is the hard floor (~32us for 4.2M logits at 1/lane/cycle);
    the schedule keeps ACT fed from ~6us: load order is IT0 XT0 IT1 XT1
    XT2 XT3 mask(0:4) IT2 IT3 mask(4:16), with k-projections for the back
    half split out (they only need x^T) so score pairs 4-7 q-half-0 can run
    before I^T finishes loading.
  * q/k projections col-tiled: lhsT=[Wq|Wk] as two concurrent 64-col PE
    tiles (q streams I^T, k streams x^T) -> one 512-cycle slot per (nb, ei);
    one full-lane DVE copy applies both biases from a [128,1] bias vector.
  * qT/kT duplicated into the opposite partition halves (DUP = [kT;qT]) via
    on-chip SBUF->SBUF DMA so score matmuls run 2x row-tiled (K=64): two ki
    chunks compute concurrently per 512-col slot.
  * exp writes straight into the sT tile; the mask multiply is an in-place
    DVE tensor_tensor (2x mode), emitted in mask-arrival order so the DVE
    stream never head-of-line blocks.
  * v projection packs 8 seq-chunks per PSUM bank (2 banks), one rank-1
    bias matmul + one DVE copy per bank.
  * ctx accumulation split per q-half so it starts as soon as the first
    half of each sT row-block is masked; epilogue in two halves so the
    output DMA overlaps the last ctx matmuls; output bf16, host upcasts.
"""

import math
from contextlib import ExitStack

import numpy as np

import concourse.bass as bass
import concourse.tile as tile
import concourse.mybir as mybir
from concourse import bacc
from concourse.bass_utils import run_bass_kernel_spmd

B, S, E, H = 8, 2048, 768, 64
N_CORES = 8
SC = S // 128   # 16 seq chunks
EC = E // 128   # 6 embed chunks
SCALE = 1.0 / math.sqrt(E)

F32 = mybir.dt.float32
BF16 = mybir.dt.bfloat16
U8 = mybir.dt.uint8
AF = mybir.ActivationFunctionType
ALU = mybir.AluOpType

_cache = {}


def _build_program():
    nc = bacc.Bacc("TRN2", target_bir_lowering=False, debug=False)

    dIT = nc.dram_tensor("IT", [E, S], BF16, kind="ExternalInput")
    dXT = nc.dram_tensor("XT", [E, S], BF16, kind="ExternalInput")
    dmT = nc.dram_tensor("maskT", [S, S], U8, kind="ExternalInput")
    dWqk = nc.dram_tensor("Wqk", [E, 128], BF16, kind="ExternalInput")
    dWv = nc.dram_tensor("Wv", [E, H], BF16, kind="ExternalInput")
    dbqk = nc.dram_tensor("bqk", [128, 1], F32, kind="ExternalInput")
    dbv = nc.dram_tensor("bv512", [1, 512], BF16, kind="ExternalInput")
    dout = nc.dram_tensor("out", [S, H], BF16, kind="ExternalOutput")

    with tile.TileContext(nc) as tc, ExitStack() as ctx:
        singles = ctx.enter_context(tc.tile_pool(name="singles", bufs=1))

        IT = singles.tile([128, EC, S], BF16, tag="IT")
        XT = singles.tile([128, EC, S], BF16, tag="XT")

        def load_cols(dst, src, lo, hi):
            # SWDGE (gpsimd) path: ~320GB/s on this segmented pattern where
            # HWDGE descriptor generation dribbles at ~95GB/s
            nc.gpsimd.dma_start(
                out=dst[:, :, lo:hi],
                in_=src.ap()[:, lo:hi].rearrange("(ec p) s -> p ec s", p=128),
            )

        # Arrival order == SWDGE FIFO order.  First chunks are 256 cols so
        # the first score/exp chain starts ~2us earlier; x^T 2,3 run before
        # I^T 2,3 so the k-side projections (and score pairs 4-7 half-0)
        # aren't gated on the full I^T stream; mask chunks last (their
        # multiplies are exp-gated anyway by then).
        load_cols(IT, dIT, 0, 256)
        load_cols(XT, dXT, 0, 256)
        load_cols(IT, dIT, 256, 512)
        load_cols(XT, dXT, 256, 512)
        load_cols(IT, dIT, 512, 1024)
        load_cols(XT, dXT, 512, 1024)
        load_cols(XT, dXT, 1024, 1536)
        load_cols(XT, dXT, 1536, 2048)
        load_cols(IT, dIT, 1024, 1536)
        load_cols(IT, dIT, 1536, 2048)

        # mask fully resident (no pool slot gating on the cast-DMA stream)
        masks = {}
        for mi in range(4):
            masks[mi] = singles.tile(
                [128, 4, S], BF16, name=f"mask{mi}", tag=f"mask{mi}"
            )
            nc.gpsimd.dma_start(
                out=masks[mi],
                in_=dmT.ap()[mi * 512:(mi + 1) * 512, :].rearrange(
                    "(t p) q -> p t q", p=128
                ),
            )

        ones_row = singles.tile([1, 512], BF16, tag="ones")
        nc.vector.memset(ones_row, 1.0)

        wqk_sb = singles.tile([128, EC, 128], BF16, tag="Wqk")
        nc.sync.dma_start(
            out=wqk_sb, in_=dWqk.ap().rearrange("(ec p) h -> p ec h", p=128)
        )
        wv_sb = singles.tile([128, EC, H], BF16, tag="Wv")
        nc.sync.dma_start(
            out=wv_sb, in_=dWv.ap().rearrange("(ec p) h -> p ec h", p=128)
        )
        bqk_sb = singles.tile([128, 1], F32, tag="bqk")
        nc.sync.dma_start(out=bqk_sb, in_=dbqk.ap())
        bv_sb = singles.tile([1, 512], BF16, tag="bv512")
        nc.sync.dma_start(out=bv_sb, in_=dbv.ap())

        # QK rows 0:64 = qT, rows 64:128 = kT.  DUP is the partition-swapped
        # copy (rows 0:64 = kT, 64:128 = qT) so both score row-tiles find
        # their operands at the right base partition.
        QK = singles.tile([128, S], BF16, tag="QK")
        DUP = singles.tile([128, S], BF16, tag="DUP")
        vA = singles.tile([128, SC, 66], BF16, tag="vA")
        nc.vector.memset(vA[:, :, H:H + 1], 1.0)

        sp = ctx.enter_context(tc.tile_pool(name="sp", bufs=16))
        psw = ctx.enter_context(tc.tile_pool(name="psw", bufs=2, space="PSUM"))
        outp = ctx.enter_context(tc.tile_pool(name="outp", bufs=1))

        def emit_proj(ps2, lo, ln=512):
            """Col-tiled q|k projection for columns lo:lo+ln."""
            ps = ps2.tile([128, 512], F32, tag="pqk")
            for ei in range(EC):
                # q and k are separate col-tile accumulation groups on the
                # same bank; each clears its own partition range at ei=0
                nc.tensor.matmul(
                    ps[0:64, 0:ln],
                    lhsT=wqk_sb[:, ei, 0:64],
                    rhs=IT[:, ei, lo:lo + ln],
                    start=(ei == 0),
                    stop=(ei == EC - 1),
                    skip_group_check=True,
                )
                nc.tensor.matmul(
                    ps[64:128, 0:ln],
                    lhsT=wqk_sb[:, ei, 64:128],
                    rhs=XT[:, ei, lo:lo + ln],
                    start=(ei == 0),
                    stop=(ei == EC - 1),
                    skip_group_check=True,
                )
            nc.vector.tensor_scalar(
                QK[:, lo:lo + ln], ps[:, 0:ln], bqk_sb, None, ALU.add
            )
            # partition-swapped copies via on-chip DMA (compute engines
            # cannot move data across partitions; DMA can, cheaply)
            nc.sync.dma_start(out=DUP[0:64, lo:lo + ln], in_=QK[64:128, lo:lo + ln])
            nc.sync.dma_start(out=DUP[64:128, lo:lo + ln], in_=QK[0:64, lo:lo + ln])

        def emit_halfproj(half, nb):
            """Uncoupled q-only (half=0) or k-only (half=1) projection for
            the back columns, so the k side can run before I^T arrives."""
            lo = nb * 512
            rlo = 64 * half
            ps = psw.tile([128, 1024], F32, tag="w")
            src = IT if half == 0 else XT
            for ei in range(EC):
                nc.tensor.matmul(
                    ps[rlo:rlo + 64, 0:512],
                    lhsT=wqk_sb[:, ei, rlo:rlo + 64],
                    rhs=src[:, ei, lo:lo + 512],
                    start=(ei == 0),
                    stop=(ei == EC - 1),
                )
            nc.vector.tensor_scalar(
                QK[rlo:rlo + 64, lo:lo + 512],
                ps[rlo:rlo + 64, 0:512],
                bqk_sb[rlo:rlo + 64],
                None,
                ALU.add,
            )
            nc.sync.dma_start(
                out=DUP[64 - rlo:128 - rlo, lo:lo + 512],
                in_=QK[rlo:rlo + 64, lo:lo + 512],
            )

        def emit_v_bank(psv, vb):
            """v projection for seq chunks 8*vb..8*vb+7 packed in one bank."""
            ps = psv.tile([128, 512], F32, tag="pv")
            for j in range(8):
                kb = vb * 8 + j
                for ei in range(EC):
                    nc.tensor.matmul(
                        ps[:, j * 64:(j + 1) * 64],
                        lhsT=XT[:, ei, kb * 128:(kb + 1) * 128],
                        rhs=wv_sb[:, ei, :],
                        start=(j == 0 and ei == 0),
                        stop=False,
                    )
            # one rank-1 bias matmul covers all 8 chunks (bv tiled 8x)
            nc.tensor.matmul(
                ps, lhsT=ones_row[:, 0:128], rhs=bv_sb, start=False, stop=True
            )
            nc.vector.tensor_copy(vA[:, vb * 8:(vb + 1) * 8, 0:H], ps)

        sTs = {}

        def emit_wexp(t, hh, split=False):
            """Row-tiled scores + exp for ki pair (2t, 2t+1), q-half hh.

            Two K=64 tiles run concurrently on the PE: tile (0,0) computes
            ki_a from DUP/QK partitions 0:64, tile (64,0) computes ki_b from
            partitions 64:128.  exp writes straight into the sT tiles; the
            mask multiply is emitted separately (emit_mult) in mask-arrival
            order.  split=True emits per-512-col exps so the first ACT op
            isn't gated on the second rhs chunk's projection."""
            ki_a, ki_b = 2 * t, 2 * t + 1
            qlo = hh * 1024
            if ki_a not in sTs:
                sTs[ki_a] = sp.tile([128, S], BF16, name=f"sT{ki_a}", tag="sT")
                sTs[ki_b] = sp.tile([128, S], BF16, name=f"sT{ki_b}", tag="sT")
            subs = split if split else ((0, 512), (512, 512))
            for ki, rlo in ((ki_a, 0), (ki_b, 64)):
                wp = psw.tile([128, 1024], F32, tag="w")
                src = DUP if rlo == 0 else QK
                mov = QK if rlo == 0 else DUP
                for off, ln in subs:
                    nc.tensor.matmul(
                        wp[:, off:off + ln],
                        lhsT=src[rlo:rlo + 64, ki * 128:(ki + 1) * 128],
                        rhs=mov[rlo:rlo + 64, qlo + off:qlo + off + ln],
                        start=True,
                        stop=True,
                    )
                    if split:
                        nc.scalar.activation(
                            sTs[ki][:, qlo + off:qlo + off + ln],
                            wp[:, off:off + ln],
                            AF.Exp,
                            scale=SCALE,
                        )
                if not split:
                    nc.scalar.activation(
                        sTs[ki][:, qlo:qlo + 1024], wp, AF.Exp, scale=SCALE
                    )

        def emit_mult(t, hh):
            """In-place mask multiply for ki pair (2t, 2t+1), q-half hh."""
            qlo = hh * 1024
            for ki in (2 * t, 2 * t + 1):
                nc.vector.tensor_tensor(
                    sTs[ki][:, qlo:qlo + 1024],
                    sTs[ki][:, qlo:qlo + 1024],
                    masks[ki // 4][:, ki % 4, qlo:qlo + 1024],
                    ALU.mult,
                )

        def emit_ctx(ki, qjs, ctxall):
            sT_sb = sTs[ki]
            for qj in qjs:
                nc.tensor.matmul(
                    ctxall[:, qj, 0:H + 1],
                    lhsT=sT_sb[:, qj * 128:(qj + 1) * 128],
                    rhs=vA[:, ki, 0:H + 1],
                    start=(ki == 0 and qj % 4 == 0),
                    stop=(ki == SC - 1 and qj % 4 == 3),
                )

        with tc.tile_pool(name="ps2", bufs=2, space="PSUM") as ps2, \
             tc.tile_pool(name="psv", bufs=2, space="PSUM") as psv:
            # PE warmup: ~3.5us of rank-1 streams during the initial DMA wait
            # flips the HAM clock-gate to 2.4GHz before real work arrives
            for _ in range(8):
                wt = ps2.tile([128, 512], F32, tag="pqk")
                nc.tensor.matmul(
                    wt, lhsT=ones_row[:, 0:128], rhs=ones_row, start=True, stop=True
                )
            emit_proj(ps2, 0, 256)
            emit_proj(ps2, 256, 256)
            emit_proj(ps2, 512)
            emit_wexp(0, 0, split=((0, 256), (256, 256), (512, 512)))
            emit_wexp(1, 0, split=((0, 512), (512, 512)))
            emit_v_bank(psv, 0)
            emit_wexp(2, 0)
            emit_wexp(3, 0)
            emit_halfproj(1, 2)   # kT cols 1024:2048 from x^T (arrives early)
            emit_halfproj(1, 3)
            emit_v_bank(psv, 1)
            emit_wexp(4, 0)
            emit_wexp(5, 0)
            emit_wexp(6, 0)
            emit_wexp(7, 0)

        # prologue PSUM pools closed -> 4 banks free for ctx accumulation
        psctx = ctx.enter_context(tc.tile_pool(name="psctx", bufs=1, space="PSUM"))
        ctxall = psctx.tile([128, SC, 128], F32, tag="ctxall")
        Q07 = tuple(range(8))
        Q8F = tuple(range(8, SC))

        emit_mult(0, 0)
        emit_mult(1, 0)
        emit_ctx(0, Q07, ctxall)
        emit_ctx(1, Q07, ctxall)
        emit_ctx(2, Q07, ctxall)
        emit_ctx(3, Q07, ctxall)
        emit_halfproj(0, 2)   # qT cols 1024:2048 once I^T lands
        emit_halfproj(0, 3)
        emit_wexp(0, 1)
        emit_wexp(1, 1)
        # DVE stream in mask-arrival order: masks 0:4 early, 4:8 after I^T,
        # then 8:12, 12:16; hh1 multiplies trail their exps
        emit_mult(0, 1)
        emit_ctx(0, Q8F, ctxall)
        emit_wexp(2, 1)
        emit_mult(2, 0)
        emit_mult(3, 0)
        emit_ctx(4, Q07, ctxall)
        emit_ctx(5, Q07, ctxall)
        emit_ctx(6, Q07, ctxall)
        emit_ctx(7, Q07, ctxall)
        emit_wexp(3, 1)
        emit_mult(1, 1)
        emit_ctx(1, Q8F, ctxall)
        emit_wexp(4, 1)
        emit_mult(4, 0)
        emit_ctx(8, Q07, ctxall)
        emit_ctx(9, Q07, ctxall)
        emit_mult(2, 1)
        emit_ctx(2, Q8F, ctxall)
        emit_wexp(5, 1)
        emit_mult(5, 0)
        emit_ctx(10, Q07, ctxall)
        emit_ctx(11, Q07, ctxall)
        emit_mult(6, 0)
        emit_mult(7, 0)
        emit_ctx(12, Q07, ctxall)
        emit_ctx(13, Q07, ctxall)
        emit_ctx(14, Q07, ctxall)
        emit_ctx(15, Q07, ctxall)
        emit_wexp(6, 1)
        emit_mult(3, 1)
        emit_ctx(3, Q8F, ctxall)
        emit_mult(4, 1)
        emit_ctx(8, Q8F, ctxall)
        emit_ctx(9, Q8F, ctxall)
        emit_wexp(7, 1)
        emit_mult(5, 1)
        emit_ctx(4, Q8F, ctxall)
        emit_ctx(5, Q8F, ctxall)
        emit_ctx(10, Q8F, ctxall)
        emit_ctx(11, Q8F, ctxall)
        emit_mult(6, 1)
        emit_ctx(6, Q8F, ctxall)
        emit_ctx(7, Q8F, ctxall)
        emit_ctx(12, Q8F, ctxall)
        emit_ctx(13, Q8F, ctxall)
        emit_mult(7, 1)
        emit_ctx(14, Q8F, ctxall)

        # epilogue in two halves so output DMA overlaps the last ctx matmuls
        recip_t = outp.tile([128, SC, 1], F32, tag="recip")
        o_all = outp.tile([128, SC, H], BF16, tag="o")

        def emit_epilogue(qlo, qhi):
            nc.vector.reciprocal(
                recip_t[:, qlo:qhi], ctxall[:, qlo:qhi, H:H + 1]
            )
            rb = bass.AP(
                tensor=recip_t.tensor,
                offset=recip_t.offset + qlo * recip_t.ap[1][0],
                ap=[recip_t.ap[0], [recip_t.ap[1][0], qhi - qlo], [0, H]],
            )
            nc.vector.tensor_tensor(
                o_all[:, qlo:qhi], ctxall[:, qlo:qhi, 0:H], rb, ALU.mult
            )
            nc.sync.dma_start(
                out=dout.ap()[qlo * 128:qhi * 128].rearrange(
                    "(qj p) h -> p qj h", p=128
                ),
                in_=o_all[:, qlo:qhi],
            )

        emit_epilogue(0, 8)
        emit_ctx(15, Q8F, ctxall)
        emit_epilogue(8, SC)

    nc.compile()
    return nc


def get_program():
    if "nc" not in _cache:
        _cache["nc"] = _build_program()
    return _cache["nc"]


def make_in_maps(I, x, mask, Wq, bq, Wk, bk, Wv, bv):
    import ml_dtypes

    BF = ml_dtypes.bfloat16
    I = np.asarray(I, dtype=np.float32)
    x = np.asarray(x, dtype=np.float32)
    mask = np.asarray(mask, dtype=np.int32)

    Wqk = np.concatenate(
        [np.asarray(Wq, np.float32), np.asarray(Wk, np.float32)], axis=1
    ).astype(BF)
    Wv_ = np.asarray(Wv, np.float32).astype(BF)
    bqk = np.concatenate(
        [np.asarray(bq, np.float32), np.asarray(bk, np.float32)]
    ).reshape(128, 1).astype(np.float32)
    bv512 = np.tile(np.asarray(bv, np.float32).reshape(1, H), (1, 8)).astype(BF)

    return [
        {
            "IT": np.ascontiguousarray(I[b].T).astype(BF),
            "XT": np.ascontiguousarray(x[b].T).astype(BF),
            "maskT": np.ascontiguousarray(mask[b].T).astype(np.uint8),
            "Wqk": Wqk, "Wv": Wv_, "bqk": bqk, "bv512": bv512,
        }
        for b in range(B)
    ]


def kernel(I, x, mask, Wq, bq, Wk, bk, Wv, bv):
    nc = get_program()
    in_maps = make_in_maps(I, x, mask, Wq, bq, Wk, bk, Wv, bv)
    res = run_bass_kernel_spmd(nc, in_maps, list(range(N_CORES)))
    out = np.stack([res.results[b]["out"] for b in range(B)], axis=0)
    return out.astype(np.float32)
